# revision 29
# baseline (speedup 1.0000x reference)
"""Trainium2 Bass kernel for nn_Attention_49185965473844.

Math (per example b):
    q = x @ Wq ; k = x @ Wk ; v = x @ Wv          (x: [S, D], W*: [D, D], D=32)
    A[q,k]   = sum_s q[s,q] k[s,k]  = (Wq^T G Wk)[q,k],   G = x^T x   ([32, 32])
    scores   = softmax(A, axis=1)                 (normalize down columns)
    out[q,s] = sum_k scores[q,k] v[s,k] = (M @ x^T)[q,s], M = scores @ Wv^T

So the whole problem reduces to: one Gram matrix G = x^T x per example, a
tiny 32x32 chain + softmax, and one [32,32] @ [32,S] matmul against x^T.

The kernel is HBM/DMA-bound (16 MB of unavoidable traffic per core), so the
layout is designed around the DMA and the DVE's 32x32 block transpose:

    s = 2048*g + 64*p' + j,  g in [0,4), p' in [0,32), j in [0,64)
    SBUF partition p = 32*g + p' (the TOP 7 bits of s)

  * load: nat[p, (r=j, d)] = x[64p + j, d] is x's natural row-major order:
    fully contiguous 8 KB per partition, cast fp32->fp16 in the DMA (SWDGE).
    fp16 (10-bit mantissa) keeps every PE matmul at 1 cyc/row with FWL
    weight loads; measured end-to-end rel err 7.8e-4 vs the 2e-2 gate.
  * gram: 16 accumulating fp16 [128,128] self products of column blocks;
    the diagonal 32x32 blocks sum to G.
  * the DVE 32x32 block transpose of nat IS the output-matmul rhs:
    T[(g,k), 32j + p'] = x[2048g + 64p' + j, k] - partition group g is the
    top 2 bits of s, so one SBUF->SBUF DVE op replaces all PE transposes.
  * block-diag matmul (bd columns ordered (q, g)) -> o[(q,g), (j, p')].
  * the mandatory PSUM->SBUF copy scatters (j, p') -> 64p' + j, so the
    assembled o_sb[(q,g), f] = out[q, 2048g + f] stores as ONE fully
    contiguous 1 MB DMA per example (on the otherwise idle sync queue).

The per-example work is software-pipelined so the PE never idles long
(HAM stays at 2.4 GHz): iteration i runs gram+transpose of example i, the
chain/softmax of example i-1, and the output matmuls/store of example i-2.

Sharding: pure data parallel over batch B=64 -> 8 examples per NeuronCore.
"""

import numpy as np

import concourse.bass as bass
import concourse.bacc as bacc
import concourse.tile as tile
from concourse import mybir
from concourse.bass_utils import run_bass_kernel_spmd

N_CORES = 8
B, S, D = 64, 8192, 32
PER_CORE = B // N_CORES  # 8

F32 = mybir.dt.float32
FP16 = mybir.dt.float16

N_R = 64   # s bits 0..5: rows per partition (load run = 64 rows = 8 KB)
N_P = 128  # s bits 6..12: SBUF partition


def build_nc(n_ex=PER_CORE, seq=S):
    """Build the per-core Bass program. Same program runs on all 8 cores."""
    assert seq == N_P * N_R
    nc = bacc.Bacc("TRN2", target_bir_lowering=False, debug=False)
    x_t = nc.declare_dram_parameter("x", [n_ex, seq, D], F32, isOutput=False)
    cst_t = nc.declare_dram_parameter("cst", [128, 352], F32, isOutput=False)
    out_t = nc.declare_dram_parameter("out", [n_ex, D, seq], F32, isOutput=True)

    with tile.TileContext(nc) as tc:
        with (
            tc.tile_pool(name="consts", bufs=1) as consts,
            tc.tile_pool(name="nat_pool", bufs=n_ex) as nat_pool,
            tc.tile_pool(name="trhs_pool", bufs=6) as trhs_pool,
            tc.tile_pool(name="osb_pool", bufs=4) as osb_pool,
            tc.tile_pool(name="small_pool", bufs=3) as small_pool,
            tc.tile_pool(name="gram_psum", bufs=2, space="PSUM") as gram_psum,
            tc.tile_pool(name="acc_psum", bufs=2, space="PSUM") as acc_psum,
            tc.tile_pool(name="o_psum", bufs=3, space="PSUM") as o_psum,
        ):
            # ---- constants ----
            cst_sb = consts.tile([128, 352], F32)
            nc.sync.dma_start(out=cst_sb, in_=cst_t[:, :])
            identity = cst_sb[:, 0:128]
            wv4 = cst_sb[:, 128:160]       # np.tile(Wv, (4, 1))
            wq4 = cst_sb[:, 160:192]       # np.tile(Wq, (4, 1))
            wk_sb = cst_sb[0:D, 192:224]
            # qgmask[p, 4*q + g] = 1.0 iff p//32 == g
            qgmask = cst_sb[:, 224:352]
            # Wv replicated on 4 partition blocks, PE-transposed so that
            # wvt_rep[k, 32*j + d] = Wv[d, k].
            wvt_ps = acc_psum.tile([D, 128], F32, tag="acc")
            nc.tensor.transpose(wvt_ps, wv4, identity)
            wvt_rep = consts.tile([D, 128], F32)
            nc.scalar.copy(out=wvt_rep, in_=wvt_ps)

            def load_nat(b):
                # nat[p, r, d] = x[b, 64p + r, d] cast fp32->fp16 in the
                # DMA (SWDGE): per partition one fully contiguous 8 KB read.
                nat = nat_pool.tile([128, N_R, D], FP16, tag="nat",
                                    name=f"nat_{b}")
                nc.gpsimd.dma_start(
                    out=nat,
                    in_=x_t[b].rearrange("(p r) d -> p r d", p=N_P, r=N_R),
                )
                return nat

            # All example loads are queued upfront (x is SBUF-resident for
            # the whole kernel) on the gpsimd SWDGE queue; stores ride the
            # sync HWDGE queue so load and store packets interleave at the
            # DMA engines.
            nats = [load_nat(b) for b in range(n_ex)]

            # per-example state carried across pipeline stages
            st = [dict() for _ in range(n_ex)]

            def out_mm(b2, t):
                """One output matmul o = bd @ trhs[:, 512t:] for example
                b-2; the PSUM->SBUF shuffle copy is emitted separately."""
                s2 = st[b2]
                o_ps = o_psum.tile([128, 512], F32, tag="o")
                nc.tensor.matmul(
                    o_ps, lhsT=s2["bd"],
                    rhs=s2["trhs"][:, 512 * t:512 * (t + 1)],
                )
                s2[f"o_ps{t}"] = o_ps

            def out_copy(b2, t, eng):
                """o_ps[z, 32 j2 + p'] -> o_sb[z, p', 16t + j2]."""
                s2 = st[b2]
                o_ps = s2.pop(f"o_ps{t}")
                dst = s2["o_sb"][:, :, 16 * t:16 * (t + 1)]
                src = o_ps.rearrange("z (j p) -> z p j", j=16, p=32)
                if eng == "v":
                    nc.vector.tensor_copy(out=dst, in_=src)
                else:
                    nc.scalar.copy(out=dst, in_=src)

            n_blk = (N_R * D) // 128  # 16 gram column blocks

            # Deep pipeline: each 32x32-chain step is its own stage, so
            # every PE instruction's inputs are >= 1 iteration old and the
            # in-order PE queue never blocks on a same-iteration copy.
            #   e0: gram + gram_sb + diag-gather DMAs + DVE transposes
            #   e1: t2 = sum_j D_j @ Wq  (one matmul vs tiled Wq)
            #   e2: A^T + softmax
            #   e3: M^T + bd mask-mul
            #   e4: output matmuls + shuffle copies + cast-store
            for it in range(n_ex + 5):
                e = [it - k for k in range(5)]
                ine = [0 <= v < n_ex for v in e]

                # ---- e0: PE gram; ACT gram_sb; DVE transposes ----
                if ine[0]:
                    b = e[0]
                    nat2 = nats[b].rearrange("p r d -> p (r d)")
                    st[b]["nat2"] = nat2
                    gram_ps = gram_psum.tile([128, 128], F32, tag="gram",
                                             name=f"gram_{b}")
                    for t in range(n_blk):
                        nc.tensor.matmul(
                            gram_ps,
                            lhsT=nat2[:, 128 * t:128 * (t + 1)],
                            rhs=nat2[:, 128 * t:128 * (t + 1)],
                            start=(t == 0),
                            stop=(t == n_blk - 1),
                        )
                    gram_sb = small_pool.tile([128, 128], F32, tag="gram_sb")
                    nc.scalar.copy(out=gram_sb, in_=gram_ps)
                    # gather the 4 diagonal 32x32 blocks into
                    # gstack[(j,e'), r] = D_j[e', r] (same partitions,
                    # column shift only): tiny SBUF->SBUF DMAs, sync queue
                    gstack = small_pool.tile([128, D], F32, tag="gstack")
                    for j in range(4):
                        sl = slice(32 * j, 32 * (j + 1))
                        nc.sync.dma_start(out=gstack[sl, :],
                                          in_=gram_sb[sl, sl])
                    st[b]["gstack"] = gstack
                    trhs = trhs_pool.tile([128, 2048], FP16, tag="trhs",
                                          name=f"trhs_{b}")
                    st[b]["trhs"] = trhs
                    for h in range(4):
                        nc.vector.transpose(
                            out=trhs[:, 512 * h:512 * (h + 1)],
                            in_=nat2[:, 512 * h:512 * (h + 1)],
                        )

                # ---- e1: t2 = G @ Wq = sum_j D_j @ Wq in ONE matmul
                # (lhsT = stacked blocks, rhs = tile(Wq,(4,1))) ----
                if ine[1]:
                    b = e[1]
                    t2_ps = acc_psum.tile([D, D], F32, tag="acc")
                    nc.tensor.matmul(t2_ps, lhsT=st[b].pop("gstack"),
                                     rhs=wq4)
                    t2_sb = small_pool.tile([D, D], F32, tag="t2_sb")
                    nc.vector.tensor_copy(out=t2_sb, in_=t2_ps)
                    st[b]["t2_sb"] = t2_sb

                # ---- e2: A^T = Wk^T t2; softmax over q (free dim) ----
                if ine[2]:
                    b = e[2]
                    at_ps = acc_psum.tile([D, D], F32, tag="acc")
                    nc.tensor.matmul(at_ps, lhsT=wk_sb,
                                     rhs=st[b].pop("t2_sb"))
                    nmax = small_pool.tile([D, 1], F32, tag="nmax")
                    nc.vector.reduce_max(
                        out=nmax, in_=at_ps, axis=mybir.AxisListType.X,
                        negate=True,
                    )
                    e_sb = small_pool.tile([D, D], F32, tag="e_sb")
                    rsum = small_pool.tile([D, 1], F32, tag="rsum")
                    nc.scalar.activation(
                        out=e_sb, in_=at_ps,
                        func=mybir.ActivationFunctionType.Exp,
                        bias=nmax, scale=1.0,
                        accum_out=rsum,
                    )
                    rinv = small_pool.tile([D, 1], F32, tag="rinv")
                    nc.vector.reciprocal(out=rinv, in_=rsum)
                    sc_sb = small_pool.tile([D, D], F32, tag="sc_sb")
                    nc.scalar.activation(
                        out=sc_sb, in_=e_sb,
                        func=mybir.ActivationFunctionType.Copy,
                        scale=rinv,
                    )
                    st[b]["sc_sb"] = sc_sb

                # ---- e3: M^T; bd mask-mul (Pool) casts to fp16 ----
                if ine[3]:
                    b = e[3]
                    m4_ps = acc_psum.tile([128, D], F32, tag="acc")
                    nc.tensor.matmul(m4_ps, lhsT=wvt_rep,
                                     rhs=st[b].pop("sc_sb"))
                    m4_sb = small_pool.tile([128, D], F32, tag="m4_sb")
                    nc.scalar.copy(out=m4_sb, in_=m4_ps)
                    bd = small_pool.tile([128, 128], FP16, tag="bd")
                    m4_bcast = bass.AP(
                        tensor=m4_sb.tensor,
                        offset=m4_sb.offset,
                        ap=[list(m4_sb.ap[0]), list(m4_sb.ap[1]), [0, 4]],
                    )
                    nc.gpsimd.tensor_mul(
                        out=bd.rearrange("p (q g) -> p q g", g=4),
                        in0=m4_bcast,
                        in1=qgmask.rearrange("p (q g) -> p q g", g=4),
                    )
                    st[b]["bd"] = bd

                # ---- e4: output matmuls + shuffle copies + cast-store
                # (o_sb kept fp16: halves the copy cost; the SWDGE store
                # casts fp16->fp32 inline; rel err stays ~9e-4) ----
                if ine[4]:
                    b = e[4]
                    st[b]["o_sb"] = osb_pool.tile(
                        [128, 32, N_R], FP16, tag="o_sb", name=f"osb_{b}"
                    )
                    for t in range(4):
                        out_mm(b, t)
                        out_copy(b, t, "v" if t % 2 == 0 else "s")
                    nc.gpsimd.dma_start(
                        out=out_t[b].rearrange("q (c f) -> (q c) f", c=4),
                        in_=st[b]["o_sb"].rearrange("z p l -> z (p l)"),
                    )

    nc.compile()
    return nc


_CACHED_NC = None


def _get_nc():
    global _CACHED_NC
    if _CACHED_NC is None:
        _CACHED_NC = build_nc()
    return _CACHED_NC


def make_cst(wq, wk, wv):
    """[128, 352]: identity | tile(Wv,(4,1)) | tile(Wq,(4,1)) | Wk | mask."""
    cst = np.zeros((128, 352), dtype=np.float32)
    cst[:, 0:128] = np.eye(128, dtype=np.float32)
    cst[:, 128:160] = np.tile(wv, (4, 1))
    cst[:, 160:192] = np.tile(wq, (4, 1))
    cst[0:D, 192:224] = wk
    pblk = np.arange(128) // 32
    g = np.arange(128) % 4
    cst[:, 224:352] = (pblk[:, None] == g[None, :]).astype(np.float32)
    return cst


def kernel(x, Wq, Wk, Wv):
    x = np.ascontiguousarray(np.asarray(x, dtype=np.float32))
    wq = np.asarray(Wq, dtype=np.float32).reshape(D, D)
    wk = np.asarray(Wk, dtype=np.float32).reshape(D, D)
    wv = np.asarray(Wv, dtype=np.float32).reshape(D, D)
    assert x.shape == (B, S, D)
    cst = make_cst(wq, wk, wv)

    nc = _get_nc()
    in_maps = [
        {
            "x": x[c * PER_CORE:(c + 1) * PER_CORE],
            "cst": cst,
        }
        for c in range(N_CORES)
    ]
    res = run_bass_kernel_spmd(nc, in_maps, list(range(N_CORES)))
    out = np.concatenate([res.results[c]["out"] for c in range(N_CORES)], axis=0)
    return out


# revision 30
# speedup vs baseline: 1.1462x; 1.1462x over previous
"""Trainium2 Bass kernel for nn_Attention_49185965473844.

Math (per example b):
    q = x @ Wq ; k = x @ Wk ; v = x @ Wv          (x: [S, D], W*: [D, D], D=32)
    A[q,k]   = sum_s q[s,q] k[s,k]  = (Wq^T G Wk)[q,k],   G = x^T x   ([32, 32])
    scores   = softmax(A, axis=1)                 (normalize down columns)
    out[q,s] = sum_k scores[q,k] v[s,k] = (M @ x^T)[q,s], M = scores @ Wv^T

So the whole problem reduces to: one Gram matrix G = x^T x per example, a
tiny 32x32 chain + softmax, and one [32,32] @ [32,S] matmul against x^T.

The kernel is HBM/DMA-bound (16 MB of unavoidable traffic per core), so the
layout is designed around the DMA and the DVE's 32x32 block transpose:

    s = 2048*g + 64*p' + j,  g in [0,4), p' in [0,32), j in [0,64)
    SBUF partition p = 32*g + p' (the TOP 7 bits of s)

  * load: nat[p, (r=j, d)] = x[64p + j, d] is x's natural row-major order:
    fully contiguous 8 KB per partition, cast fp32->fp16 in the DMA (SWDGE).
    fp16 (10-bit mantissa) keeps every PE matmul at 1 cyc/row with FWL
    weight loads; measured end-to-end rel err 7.8e-4 vs the 2e-2 gate.
  * gram: 16 accumulating fp16 [128,128] self products of column blocks;
    the diagonal 32x32 blocks sum to G.
  * the DVE 32x32 block transpose of nat IS the output-matmul rhs:
    T[(g,k), 32j + p'] = x[2048g + 64p' + j, k] - partition group g is the
    top 2 bits of s, so one SBUF->SBUF DVE op replaces all PE transposes.
  * block-diag matmul (bd columns ordered (q, g)) -> o[(q,g), (j, p')].
  * the mandatory PSUM->SBUF copy scatters (j, p') -> 64p' + j, so the
    assembled o_sb[(q,g), f] = out[q, 2048g + f] stores as ONE fully
    contiguous 1 MB DMA per example (on the otherwise idle sync queue).

The per-example work is software-pipelined so the PE never idles long
(HAM stays at 2.4 GHz): iteration i runs gram+transpose of example i, the
chain/softmax of example i-1, and the output matmuls/store of example i-2.

Sharding: pure data parallel over batch B=64 -> 8 examples per NeuronCore.
"""

import numpy as np

import concourse.bass as bass
import concourse.bacc as bacc
import concourse.tile as tile
from concourse import mybir
from concourse.bass_utils import run_bass_kernel_spmd

N_CORES = 8
B, S, D = 64, 8192, 32
PER_CORE = B // N_CORES  # 8

F32 = mybir.dt.float32
FP16 = mybir.dt.float16

N_R = 64   # s bits 0..5: rows per partition (load run = 64 rows = 8 KB)
N_P = 128  # s bits 6..12: SBUF partition


def build_nc(n_ex=PER_CORE, seq=S):
    """Build the per-core Bass program. Same program runs on all 8 cores."""
    assert seq == N_P * N_R
    nc = bacc.Bacc("TRN2", target_bir_lowering=False, debug=False)
    x_t = nc.declare_dram_parameter("x", [n_ex, seq, D], F32, isOutput=False)
    cst_t = nc.declare_dram_parameter("cst", [128, 352], F32, isOutput=False)
    out_t = nc.declare_dram_parameter("out", [n_ex, D, seq], F32, isOutput=True)

    with tile.TileContext(nc) as tc:
        with (
            tc.tile_pool(name="consts", bufs=1) as consts,
            tc.tile_pool(name="nat_pool", bufs=n_ex) as nat_pool,
            tc.tile_pool(name="trhs_pool", bufs=6) as trhs_pool,
            tc.tile_pool(name="osb_pool", bufs=4) as osb_pool,
            tc.tile_pool(name="small_pool", bufs=3) as small_pool,
            tc.tile_pool(name="gram_psum", bufs=2, space="PSUM") as gram_psum,
            tc.tile_pool(name="acc_psum", bufs=2, space="PSUM") as acc_psum,
            tc.tile_pool(name="o_psum", bufs=3, space="PSUM") as o_psum,
        ):
            # ---- constants ----
            cst_sb = consts.tile([128, 352], F32)
            nc.sync.dma_start(out=cst_sb, in_=cst_t[:, :])
            identity = cst_sb[:, 0:128]
            wv4 = cst_sb[:, 128:160]       # np.tile(Wv, (4, 1))
            wq4 = cst_sb[:, 160:192]       # np.tile(Wq, (4, 1))
            wk_sb = cst_sb[0:D, 192:224]
            # qgmask[p, 4*q + g] = 1.0 iff p//32 == g
            qgmask = cst_sb[:, 224:352]
            # Wv replicated on 4 partition blocks, PE-transposed so that
            # wvt_rep[k, 32*j + d] = Wv[d, k].
            wvt_ps = acc_psum.tile([D, 128], F32, tag="acc")
            nc.tensor.transpose(wvt_ps, wv4, identity)
            wvt_rep = consts.tile([D, 128], F32)
            nc.scalar.copy(out=wvt_rep, in_=wvt_ps)

            def load_nat(b):
                # nat[p, r, d] = x[b, 64p + r, d] cast fp32->fp16 in the
                # DMA (SWDGE): per partition one fully contiguous 8 KB read.
                nat = nat_pool.tile([128, N_R, D], FP16, tag="nat",
                                    name=f"nat_{b}")
                nc.gpsimd.dma_start(
                    out=nat,
                    in_=x_t[b].rearrange("(p r) d -> p r d", p=N_P, r=N_R),
                )
                return nat

            # All example loads are queued upfront (x is SBUF-resident for
            # the whole kernel) on the gpsimd SWDGE queue; stores ride the
            # sync HWDGE queue so load and store packets interleave at the
            # DMA engines.
            nats = [load_nat(b) for b in range(n_ex)]

            # per-example state carried across pipeline stages
            st = [dict() for _ in range(n_ex)]

            def out_mm(b2, t):
                """One output matmul o = bd @ trhs[:, 512t:] for example
                b-2; the PSUM->SBUF shuffle copy is emitted separately."""
                s2 = st[b2]
                o_ps = o_psum.tile([128, 512], F32, tag="o")
                nc.tensor.matmul(
                    o_ps, lhsT=s2["bd"],
                    rhs=s2["trhs"][:, 512 * t:512 * (t + 1)],
                )
                s2[f"o_ps{t}"] = o_ps

            def out_copy(b2, t, eng):
                """o_ps[z, 32 j2 + p'] -> o_sb[z, p', 16t + j2]."""
                s2 = st[b2]
                o_ps = s2.pop(f"o_ps{t}")
                dst = s2["o_sb"][:, :, 16 * t:16 * (t + 1)]
                src = o_ps.rearrange("z (j p) -> z p j", j=16, p=32)
                if eng == "v":
                    nc.vector.tensor_copy(out=dst, in_=src)
                else:
                    nc.scalar.copy(out=dst, in_=src)

            n_blk = (N_R * D) // 128  # 16 gram column blocks

            # Deep pipeline: each 32x32-chain step is its own stage, so
            # every PE instruction's inputs are >= 1 iteration old and the
            # in-order PE queue never blocks on a same-iteration copy.
            #   e0: gram + gram_sb + diag-gather DMAs + DVE transposes
            #   e1: t2 = sum_j D_j @ Wq  (one matmul vs tiled Wq)
            #   e2: A^T + softmax
            #   e3: M^T + bd mask-mul
            #   e4: output matmuls + shuffle copies + cast-store
            for it in range(n_ex + 5):
                e = [it - k for k in range(5)]
                ine = [0 <= v < n_ex for v in e]

                # ---- e0: PE gram; ACT gram_sb; DVE transposes ----
                if ine[0]:
                    b = e[0]
                    nat2 = nats[b].rearrange("p r d -> p (r d)")
                    st[b]["nat2"] = nat2
                    gram_ps = gram_psum.tile([128, 128], F32, tag="gram",
                                             name=f"gram_{b}")
                    for t in range(n_blk):
                        nc.tensor.matmul(
                            gram_ps,
                            lhsT=nat2[:, 128 * t:128 * (t + 1)],
                            rhs=nat2[:, 128 * t:128 * (t + 1)],
                            start=(t == 0),
                            stop=(t == n_blk - 1),
                        )
                    gram_sb = small_pool.tile([128, 128], F32, tag="gram_sb")
                    nc.scalar.copy(out=gram_sb, in_=gram_ps)
                    # gather the 4 diagonal 32x32 blocks into
                    # gstack[(j,e'), r] = D_j[e', r] (same partitions,
                    # column shift only): tiny SBUF->SBUF DMAs, sync queue
                    gstack = small_pool.tile([128, D], F32, tag="gstack")
                    for j in range(4):
                        sl = slice(32 * j, 32 * (j + 1))
                        nc.sync.dma_start(out=gstack[sl, :],
                                          in_=gram_sb[sl, sl])
                    st[b]["gstack"] = gstack
                    trhs = trhs_pool.tile([128, 2048], FP16, tag="trhs",
                                          name=f"trhs_{b}")
                    st[b]["trhs"] = trhs
                    for h in range(4):
                        nc.vector.transpose(
                            out=trhs[:, 512 * h:512 * (h + 1)],
                            in_=nat2[:, 512 * h:512 * (h + 1)],
                        )

                # ---- e1: t2 = G @ Wq = sum_j D_j @ Wq in ONE matmul
                # (lhsT = stacked blocks, rhs = tile(Wq,(4,1))) ----
                if ine[1]:
                    b = e[1]
                    t2_ps = acc_psum.tile([D, D], F32, tag="acc")
                    nc.tensor.matmul(t2_ps, lhsT=st[b].pop("gstack"),
                                     rhs=wq4)
                    t2_sb = small_pool.tile([D, D], F32, tag="t2_sb")
                    nc.vector.tensor_copy(out=t2_sb, in_=t2_ps)
                    st[b]["t2_sb"] = t2_sb

                # ---- e2: A^T = Wk^T t2; softmax over q (free dim) ----
                if ine[2]:
                    b = e[2]
                    at_ps = acc_psum.tile([D, D], F32, tag="acc")
                    nc.tensor.matmul(at_ps, lhsT=wk_sb,
                                     rhs=st[b].pop("t2_sb"))
                    nmax = small_pool.tile([D, 1], F32, tag="nmax")
                    nc.vector.reduce_max(
                        out=nmax, in_=at_ps, axis=mybir.AxisListType.X,
                        negate=True,
                    )
                    e_sb = small_pool.tile([D, D], F32, tag="e_sb")
                    rsum = small_pool.tile([D, 1], F32, tag="rsum")
                    nc.scalar.activation(
                        out=e_sb, in_=at_ps,
                        func=mybir.ActivationFunctionType.Exp,
                        bias=nmax, scale=1.0,
                        accum_out=rsum,
                    )
                    rinv = small_pool.tile([D, 1], F32, tag="rinv")
                    nc.vector.reciprocal(out=rinv, in_=rsum)
                    sc_sb = small_pool.tile([D, D], F32, tag="sc_sb")
                    nc.scalar.activation(
                        out=sc_sb, in_=e_sb,
                        func=mybir.ActivationFunctionType.Copy,
                        scale=rinv,
                    )
                    st[b]["sc_sb"] = sc_sb

                # ---- e3: M^T; bd mask-mul (Pool) casts to fp16 ----
                if ine[3]:
                    b = e[3]
                    m4_ps = acc_psum.tile([128, D], F32, tag="acc")
                    nc.tensor.matmul(m4_ps, lhsT=wvt_rep,
                                     rhs=st[b].pop("sc_sb"))
                    m4_sb = small_pool.tile([128, D], F32, tag="m4_sb")
                    nc.scalar.copy(out=m4_sb, in_=m4_ps)
                    bd = small_pool.tile([128, 128], FP16, tag="bd")
                    m4_bcast = bass.AP(
                        tensor=m4_sb.tensor,
                        offset=m4_sb.offset,
                        ap=[list(m4_sb.ap[0]), list(m4_sb.ap[1]), [0, 4]],
                    )
                    nc.gpsimd.tensor_mul(
                        out=bd.rearrange("p (q g) -> p q g", g=4),
                        in0=m4_bcast,
                        in1=qgmask.rearrange("p (q g) -> p q g", g=4),
                    )
                    st[b]["bd"] = bd

                # ---- e4: output matmuls + shuffle copies + store ----
                if ine[4]:
                    b = e[4]
                    st[b]["o_sb"] = osb_pool.tile(
                        [128, 32, N_R], F32, tag="o_sb", name=f"osb_{b}"
                    )
                    for t in range(4):
                        out_mm(b, t)
                        out_copy(b, t, "v" if t % 2 == 0 else "s")
                    nc.sync.dma_start(
                        out=out_t[b].rearrange("q (c f) -> (q c) f", c=4),
                        in_=st[b]["o_sb"].rearrange("z p l -> z (p l)"),
                    )

    nc.compile()
    return nc


_CACHED_NC = None


def _get_nc():
    global _CACHED_NC
    if _CACHED_NC is None:
        _CACHED_NC = build_nc()
    return _CACHED_NC


def make_cst(wq, wk, wv):
    """[128, 352]: identity | tile(Wv,(4,1)) | tile(Wq,(4,1)) | Wk | mask."""
    cst = np.zeros((128, 352), dtype=np.float32)
    cst[:, 0:128] = np.eye(128, dtype=np.float32)
    cst[:, 128:160] = np.tile(wv, (4, 1))
    cst[:, 160:192] = np.tile(wq, (4, 1))
    cst[0:D, 192:224] = wk
    pblk = np.arange(128) // 32
    g = np.arange(128) % 4
    cst[:, 224:352] = (pblk[:, None] == g[None, :]).astype(np.float32)
    return cst


def kernel(x, Wq, Wk, Wv):
    x = np.ascontiguousarray(np.asarray(x, dtype=np.float32))
    wq = np.asarray(Wq, dtype=np.float32).reshape(D, D)
    wk = np.asarray(Wk, dtype=np.float32).reshape(D, D)
    wv = np.asarray(Wv, dtype=np.float32).reshape(D, D)
    assert x.shape == (B, S, D)
    cst = make_cst(wq, wk, wv)

    nc = _get_nc()
    in_maps = [
        {
            "x": x[c * PER_CORE:(c + 1) * PER_CORE],
            "cst": cst,
        }
        for c in range(N_CORES)
    ]
    res = run_bass_kernel_spmd(nc, in_maps, list(range(N_CORES)))
    out = np.concatenate([res.results[c]["out"] for c in range(N_CORES)], axis=0)
    return out
